# revision 35
# baseline (speedup 1.0000x reference)
"""Trainium2 Bass kernel v2 for nn_EnhancedQuantumLayer (10-qubit, 4-layer
variational circuit, batch 512, Z-expectations output).

Scheme (data parallel over 8 cores, 64 samples/core):
  - Realified complex: re/im is a partition bit c. Layout A partitions
    p = (c, q5, q0..q4), free f = (b4..b1, b0, q6..q9) per half (b5).
    A 32x32 DVE block transpose flips to layout B p = (c, q5, b0, q6..q9),
    f = (b4..b1, q0..q4). One bf16 matmul per (layer, side, half).
  - Feature map: v(t) per (sample, qubit) is an exact 94-term Fourier
    series in t = tanh(x) (odd multiples of 0.25). Built on-device via
    partition_broadcast + per-partition tensor_scalar + range reduce +
    one ACT Sin + one PE contraction; un-transposed by 10 tiny PE
    transposes.
  - Initial state X = (h6 outer l4) built by 2 accumulated matmuls per
    half with the per-sample h6 amplitudes as stationary (k = samples).
  - Measurement: squares + two sign-weight matmuls, all in layout B.

Host precompute is theta-only: 8 realified 128x128 stationaries (bf16).
"""

import math

import ml_dtypes
import numpy as np

N_QUBITS = 10
N_LAYERS = 4
FREQS = (1.0, 2.0, 4.0, 8.0, 16.0)
PI = float(np.pi)
B_TOTAL = 512
B_CORE = 64
N_CORES = 8
WARMUP_MM = 8

CZCNOT = np.array([[1, 0, 0, 0],
                   [0, 1, 0, 0],
                   [0, 0, 0, -1],
                   [0, 0, 1, 0]], dtype=np.complex128)


# ---------------------------------------------------------------- host math
def _rz(phi):
    return np.array([[np.exp(-0.5j * phi), 0], [0, np.exp(0.5j * phi)]])


def _rx(th):
    c, s = np.cos(th / 2), np.sin(th / 2)
    return np.array([[c, -1j * s], [-1j * s, c]])


def _ry(th):
    c, s = np.cos(th / 2), np.sin(th / 2)
    return np.array([[c, -s], [s, c]])


def _kron_list(ms):
    out = ms[0]
    for m in ms[1:]:
        out = np.kron(out, m)
    return out


def _embed_2q(space_qubits, qa, qb, M4):
    n = len(space_qubits)
    dim = 2 ** n
    pa, pb = space_qubits.index(qa), space_qubits.index(qb)
    out = np.zeros((dim, dim), dtype=np.complex128)
    for idx in range(dim):
        bits = [(idx >> (n - 1 - i)) & 1 for i in range(n)]
        col4 = 2 * bits[pa] + bits[pb]
        for row4 in range(4):
            val = M4[row4, col4]
            if val != 0:
                nb = bits.copy()
                nb[pa], nb[pb] = row4 >> 1, row4 & 1
                ridx = sum(bit << (n - 1 - i) for i, bit in enumerate(nb))
                out[ridx, idx] += val
    return out


def _realify(M):
    return np.block([[M.real, -M.imag], [M.imag, M.real]])


def _embed_OB(M_L):
    """layout-B partition op on (q5, b0, q6..q9): identity on b0."""
    M = M_L.reshape(2, 16, 2, 16)
    O = np.zeros((2, 2, 16, 2, 2, 16), np.complex128)
    for b0 in range(2):
        O[:, b0, :, :, b0, :] = M
    return O.reshape(64, 64)


def _host_weights(theta):
    """wstack [128, 8, 128] bf16: per layer [lhsT_A, lhsT_L] with
    lhsT = realify(op).T, partition-major for one contiguous DMA."""
    ang = np.tanh(theta.astype(np.float64)) * PI
    A_space = [5, 0, 1, 2, 3, 4]
    L_space = [5, 6, 7, 8, 9]
    mats = []
    for l in range(N_LAYERS):
        U = []
        for q in range(10):
            a0, a1, a2 = ang[l, q]
            U.append(_rx(a0 * 0.5) @ _rz(a2) @ _ry(a1) @ _rz(a0))
        UA = _kron_list([U[q] for q in A_space])
        E_even_A = (_embed_2q(A_space, 0, 1, CZCNOT)
                    @ _embed_2q(A_space, 2, 3, CZCNOT)
                    @ _embed_2q(A_space, 4, 5, CZCNOT))
        E_odd_A = (_embed_2q(A_space, 1, 2, CZCNOT)
                   @ _embed_2q(A_space, 3, 4, CZCNOT))
        M_A = E_odd_A @ E_even_A @ UA
        UL = _kron_list([np.eye(2)] + [U[q] for q in [6, 7, 8, 9]])
        E_even_L = (_embed_2q(L_space, 6, 7, CZCNOT)
                    @ _embed_2q(L_space, 8, 9, CZCNOT))
        E_odd_L = (_embed_2q(L_space, 5, 6, CZCNOT)
                   @ _embed_2q(L_space, 7, 8, CZCNOT))
        M_L = E_odd_L @ E_even_L @ UL
        mats.append(_realify(M_A).T)
        mats.append(_realify(_embed_OB(M_L)).T)
    stk = np.stack(mats)  # [8, 128, 128]
    return np.ascontiguousarray(
        stk.transpose(1, 0, 2).astype(ml_dtypes.bfloat16))


# ------------------------------------------------------- fourier basis (v)
def _v_of_t(t):
    t = np.atleast_1d(np.asarray(t, np.float64))
    v = np.zeros((t.size, 2), np.complex128)
    v[:, 0] = 1.0
    for f in FREQS:
        phi = f * t
        v = v * np.stack([np.exp(-0.5j * phi), np.exp(0.5j * phi)], -1)
        th = 0.25 * f * t
        c, s = np.cos(th), np.sin(th)
        v = np.stack([c * v[:, 0] - 1j * s * v[:, 1],
                      -1j * s * v[:, 0] + c * v[:, 1]], -1)
    return v


def _fourier_C():
    """C [94, 4]: rows 0-46 sin(0.25 m t), rows 47-93 cos, m = 1,3..93;
    comps (ar, ai, br, bi)."""
    N = 1024
    ts = np.arange(N) * (8 * np.pi / N)
    vv = _v_of_t(ts)
    comps = np.stack([vv[:, 0].real, vv[:, 0].imag,
                      vv[:, 1].real, vv[:, 1].imag], -1)
    F = np.fft.rfft(comps, axis=0)
    msk = np.arange(1, 94, 2)
    a_cos = 2.0 * F[msk].real / N
    b_sin = -2.0 * F[msk].imag / N
    return msk, np.concatenate([b_sin, a_cos], 0).astype(np.float64)


# ------------------------------------------------------------- bass builder
_BUILD_CACHE = {}


def _measurement_weights():
    # W1 [128, 32]: col = b0p*16 + o; o: 0 = one, 1..5 = s5..s9
    W1 = np.zeros((128, 32), np.float32)
    for p in range(128):
        q5 = (p >> 5) & 1
        b0 = (p >> 4) & 1
        j4 = p & 15
        s = [1 - 2 * q5] + [1 - 2 * ((j4 >> (3 - k)) & 1) for k in range(4)]
        W1[p, b0 * 16 + 0] = 1.0
        for k in range(5):
            W1[p, b0 * 16 + 1 + k] = s[k]
    # W2 [32, 8]: m 0 = ones, 1..5 = sg_q0..sg_q4 over j5' = (q0..q4)
    W2 = np.zeros((32, 8), np.float32)
    for j in range(32):
        W2[j, 0] = 1.0
        for q in range(5):
            W2[j, 1 + q] = 1 - 2 * ((j >> (4 - q)) & 1)
    return W1, W2


def _build_module():
    import concourse.bass as bass
    import concourse.mybir as mybir
    from concourse import bacc
    from concourse.tile import TileContext

    f32 = mybir.dt.float32
    f32r = mybir.dt.float32r
    bf16 = mybir.dt.bfloat16
    AF = mybir.ActivationFunctionType
    OP = mybir.AluOpType

    nc = bacc.Bacc("TRN2", target_bir_lowering=False, debug=False)

    xin = nc.dram_tensor("xin", [B_CORE, 10], f32, kind="ExternalInput").ap()
    wstack = nc.dram_tensor("wstack", [128, 8, 128], bf16,
                            kind="ExternalInput").ap()
    out_d = nc.dram_tensor("out", [B_CORE, 10], f32, kind="ExternalOutput").ap()

    # ---- inline constants
    msk, C94 = _fourier_C()
    cpf = np.zeros((128, 16), np.float32)  # f32 consts: C, I4, signs
    cpf[0:94, 2:6] = C94
    cpf[0:4, 6:10] = np.eye(4)
    # scol [64, 2] signs; umask [64, 32]
    cpf[0:64, 10] = -1.0
    cpf[0:64, 11] = 1.0
    # cpw [2, 128]: rows (omega_m, bias_m) for the args matmul stationary
    cpw = np.zeros((2, 128), np.float32)
    cpw[0, 0:47] = 0.25 * msk
    cpw[0, 47:94] = 0.25 * msk
    cpw[1, 47:94] = 0.5 * PI
    cpm = np.zeros((64, 32), np.float32)
    for b in range(64):
        cpm[b, b % 32] = 1.0
    W1, W2 = _measurement_weights()
    cph = np.zeros((128, 32 + 8), np.float32)
    cph[:, 0:32] = W1
    cph[0:32, 32:40] = W2
    cpf_c = nc.inline_tensor(cpf, name="cpf").ap()
    cpw_c = nc.inline_tensor(cpw, name="cpw").ap()
    ones_c = nc.inline_tensor(np.ones((1, 640), np.float32), name="ones1").ap()
    cpm_c = nc.inline_tensor(cpm, name="cpm").ap()
    cph_c = nc.inline_tensor(cph.astype(ml_dtypes.bfloat16), name="cph").ap()

    MAGIC = 1.5 * 2 ** 23
    TWO_PI = 2.0 * PI
    PCLAMP = PI * (1.0 - 1e-6)

    with TileContext(nc) as tc:
        with (
            tc.tile_pool(name="wpool", bufs=1) as wpool,
            tc.tile_pool(name="sm", bufs=2) as sm,
            tc.tile_pool(name="dbl", bufs=2) as db,
            tc.tile_pool(name="xp", bufs=4) as xp,
            tc.tile_pool(name="cv", bufs=4) as cv,
            tc.tile_pool(name="psA", bufs=1, space="PSUM") as psA,
            tc.tile_pool(name="psB", bufs=1, space="PSUM") as psB,
            tc.tile_pool(name="psS", bufs=1, space="PSUM") as psS,
            tc.tile_pool(name="psW", bufs=1, space="PSUM") as psW,
        ):
            # ---- DMAs: xq first (critical path), weights on other queues
            # xq: [1, 640] (q-major flatten of xin, transposed DMA)
            xq = sm.tile([1, 640], f32, tag="xq")
            nc.sync.dma_start(
                xq[:].rearrange("p (q b) -> p q b", b=64),
                xin.rearrange("b q -> q b").unsqueeze(0))
            ctf = wpool.tile([128, 16], f32, tag="cpf")
            nc.sync.dma_start(ctf[:], cpf_c)
            ctw = wpool.tile([2, 128], f32, tag="cpw")
            nc.sync.dma_start(ctw[:], cpw_c)

            wt = wpool.tile([128, 8 * 128], bf16, tag="w")
            nc.scalar.dma_start(
                wt[:].rearrange("p (m c) -> p m c", c=128), wstack)
            ctm = wpool.tile([64, 32], f32, tag="cpm")
            nc.scalar.dma_start(ctm[:], cpm_c)
            cth = wpool.tile([128, 40], bf16, tag="cph")
            nc.scalar.dma_start(cth[:], cph_c)

            cmat = ctf[:, 2:6]
            ident4 = ctf[0:4, 6:10]
            scol = ctf[0:64, 10:12]
            umask = ctm[:]
            w1_t = cth[:, 0:32]
            w2_t = cth[0:32, 32:40]

            def W(m):
                return wt[:, 128 * m:128 * m + 128]

            # dummy Silu first: steers the act-table pass to load the one
            # table (silu_and_others) that serves Tanh+Sin+Copy+Square too
            sdum = sm.tile([1, 1], f32, tag="sdum")
            nc.scalar.activation(sdum[:], ctf[0:1, 0:1], AF.Silu)

            # ---- feature map: t on one row, args via PE, range-reduce, sin
            tmov = sm.tile([2, 640], f32, tag="tmov")
            nc.sync.dma_start(tmov[1:2, :], ones_c)
            nc.scalar.activation(tmov[0:1, :].bitcast(f32r), xq[:], AF.Tanh)
            argp = [None, None]
            for i in range(2):
                ap_ = psA.tile([128, 320], f32, tag=f"y{i}")
                nc.tensor.matmul(ap_[:], ctw[:].bitcast(f32r),
                                 tmov[:, 320 * i:320 * (i + 1)].bitcast(f32r),
                                 start=True, stop=True)
                argp[i] = ap_

            # ---- PE warmup chain (keeps HAM hot until the layer loop)
            wscr = psW.tile([128, 512], f32, tag="warm")
            for i in range(WARMUP_MM):
                nc.tensor.matmul(wscr[:], W(0), wt[:, 0:512],
                                 start=True, stop=True,
                                 skip_group_check=True)

            trig = sm.tile([128, 640], f32, tag="trig")
            for i in range(2):
                sl = slice(320 * i, 320 * (i + 1))
                kk = sm.tile([128, 320], f32, tag=f"kk{i}")
                nc.vector.tensor_scalar(kk[:], argp[i][:], 1.0 / TWO_PI,
                                        MAGIC, OP.mult, OP.add)
                nc.vector.tensor_scalar(kk[:], kk[:], MAGIC, None,
                                        OP.subtract)
                ar = sm.tile([128, 320], f32, tag=f"ar{i}")
                nc.vector.scalar_tensor_tensor(ar[:], kk[:], -TWO_PI,
                                               argp[i][:], OP.mult, OP.add)
                nc.vector.tensor_scalar(ar[:], ar[:], PCLAMP, -PCLAMP,
                                        OP.min, OP.max)
                nc.scalar.activation(trig[:, sl].bitcast(f32r), ar[:],
                                     AF.Sin)

            # ---- v = C^T trig : 2 matmuls then 10 tiny PE transposes
            vp0 = psS.tile([4, 320], f32, tag="vp0")
            vp1 = psS.tile([4, 320], f32, tag="vp1")
            vps = [vp0, vp1]
            for i in range(2):
                nc.tensor.matmul(vps[i][:], cmat.bitcast(f32r),
                                 trig[:, 320 * i:320 * (i + 1)].bitcast(f32r),
                                 start=True, stop=True)
            vsb = sm.tile([4, 640], f32, tag="vsb")
            for i in range(2):
                nc.scalar.activation(
                    vsb[:, 320 * i:320 * (i + 1)].bitcast(f32r),
                    vps[i][:], AF.Copy)
            vT = psS.tile([64, 40], f32, tag="vT")
            for q in range(10):
                nc.tensor.transpose(
                    vT[:, 4 * q:4 * q + 4].bitcast(f32r),
                    vsb[0:4, 64 * q:64 * q + 64].bitcast(f32r),
                    ident4.bitcast(f32r))
            v_cur = sm.tile([64, 40], f32, tag="vcur")
            nc.vector.tensor_copy(v_cur[:], vT[:])

            # ---- doubling: h6 = v5 x v0..v4 (j6 idx), l4 = v6 x..x v9
            vv = v_cur[:].rearrange("p (q a c) -> p q a c", a=2, c=2)
            vimS = sm.tile([64, 40], f32, tag="vimS")
            vimS_v = vimS[:].rearrange("p (q a c) -> p q a c", a=2, c=2)
            nc.vector.tensor_tensor(
                vimS_v,
                vv[:, :, :, 1:2].broadcast_to((64, 10, 2, 2)),
                scol.unsqueeze(1).unsqueeze(1)
                    .broadcast_to((64, 10, 2, 2))
                    .rearrange("p q a c -> p q a c"),
                OP.mult)

            def vre(q):
                return vv[:, q, :, 0]  # [64, 2]

            def vim_s(q):
                return vimS_v[:, q]  # [64, 2, 2]

            def kstep(eng, Xt, m, Yre, YimS, n, tag, dtype=f32, pool=db):
                """out[p, (j, a, c)] = complex (X kron Y); X [64, m*2]."""
                Xv = Xt[:].rearrange("p (m c) -> p m c", c=2)
                t1 = pool.tile([64, m * n * 2], f32, tag=tag + "1")
                t2 = pool.tile([64, m * n * 2], f32, tag=tag + "2")
                out = pool.tile([64, m * n * 2], dtype, tag=tag)
                t1v = t1[:].rearrange("p (m n c) -> p m n c", m=m, c=2)
                t2v = t2[:].rearrange("p (m n c) -> p m n c", m=m, c=2)
                Xb = Xv.unsqueeze(2).broadcast_to((64, m, n, 2))
                Xsw = (Xv[:, :, ::-1].unsqueeze(2)
                       .broadcast_to((64, m, n, 2)))
                Yreb = (Yre.unsqueeze(1).unsqueeze(3)
                        .broadcast_to((64, m, n, 2)))
                YimSb = YimS.unsqueeze(1).broadcast_to((64, m, n, 2))
                eng.tensor_tensor(t1v, Xb, Yreb, OP.mult)
                eng.tensor_tensor(t2v, Xsw, YimSb, OP.mult)
                eng.tensor_tensor(out[:], t1[:], t2[:], OP.add)
                return out

            def mk_imS(eng, Yt, n, tag, pool=db):
                o = pool.tile([64, n * 2], f32, tag=tag)
                ov = o[:].rearrange("p (n c) -> p n c", c=2)
                Yv = Yt[:].rearrange("p (n c) -> p n c", c=2)
                eng.tensor_tensor(
                    ov, Yv[:, :, 1:2].broadcast_to((64, n, 2)),
                    scol.unsqueeze(1).broadcast_to((64, n, 2)),
                    OP.mult)
                return o

            V = nc.vector
            # all chains on DVE (GPSIMD semaphore wake-up is ~5us; keep it
            # entirely off the critical path)
            v5t = sm.tile([64, 4], f32, tag="v5t")
            nc.vector.tensor_copy(
                v5t[:].rearrange("p (a c) -> p a c", c=2), vv[:, 5])
            a1 = kstep(V, v5t, 2, vre(0), vim_s(0), 2, "a1")
            a2 = kstep(V, a1, 4, vre(1), vim_s(1), 2, "a2")
            v6t = sm.tile([64, 4], f32, tag="v6t")
            nc.vector.tensor_copy(
                v6t[:].rearrange("p (a c) -> p a c", c=2), vv[:, 6])
            c1 = kstep(V, v6t, 2, vre(7), vim_s(7), 2, "c1")
            v2t = sm.tile([64, 4], f32, tag="v2t")
            nc.vector.tensor_copy(
                v2t[:].rearrange("p (a c) -> p a c", c=2), vv[:, 2])
            b1 = kstep(V, v2t, 2, vre(3), vim_s(3), 2, "b1")
            b2 = kstep(V, b1, 4, vre(4), vim_s(4), 2, "b2")
            b2S = mk_imS(V, b2, 8, "b2S")
            v8t = sm.tile([64, 4], f32, tag="v8t")
            nc.vector.tensor_copy(
                v8t[:].rearrange("p (a c) -> p a c", c=2), vv[:, 8])
            c2 = kstep(V, v8t, 2, vre(9), vim_s(9), 2, "c2")
            c2S = mk_imS(V, c2, 4, "c2S")
            # h6 = a2 x b2 (DVE) written c-major bf16 (stationary layout),
            # l4 = c1 x c2 (GPS)
            b2re = b2[:].rearrange("p (n c) -> p n c", c=2)[:, :, 0]
            b2Sv = b2S[:].rearrange("p (n c) -> p n c", c=2)
            h6t1 = db.tile([64, 128], f32, tag="h61")
            h6t2 = db.tile([64, 128], f32, tag="h62")
            t1v = h6t1[:].rearrange("p (m n c) -> p m n c", m=8, c=2)
            t2v = h6t2[:].rearrange("p (m n c) -> p m n c", m=8, c=2)
            a2v = a2[:].rearrange("p (m c) -> p m c", c=2)
            nc.vector.tensor_tensor(
                t1v, a2v.unsqueeze(2).broadcast_to((64, 8, 8, 2)),
                b2re.unsqueeze(1).unsqueeze(3).broadcast_to((64, 8, 8, 2)),
                OP.mult)
            nc.vector.tensor_tensor(
                t2v, a2v[:, :, ::-1].unsqueeze(2).broadcast_to((64, 8, 8, 2)),
                b2Sv.unsqueeze(1).broadcast_to((64, 8, 8, 2)),
                OP.mult)
            s1bf = sm.tile([64, 128], bf16, tag="s1bf")
            s1w = s1bf[:].rearrange("p (c m n) -> p m n c", c=2, m=8)
            nc.vector.tensor_tensor(
                s1w, t1v, t2v, OP.add)
            c2re = c2[:].rearrange("p (n c) -> p n c", c=2)[:, :, 0]
            l4 = kstep(V, c1, 4, c2re,
                       c2S[:].rearrange("p (n c) -> p n c", c=2), 4, "l4")

            # ---- S2 from S1 (c-major): S2[c] = sign(c) * S1[1-c]
            s1cm = s1bf[:].rearrange("p (c j) -> p c j", c=2)
            s2bf = sm.tile([64, 128], bf16, tag="s2bf")
            nc.vector.tensor_tensor(
                s2bf[:].rearrange("p (c j) -> p c j", c=2),
                s1cm[:, ::-1, :],
                scol.unsqueeze(2).broadcast_to((64, 2, 64)),
                OP.mult)

            l4v = l4[:].rearrange("p (j c) -> p j c", c=2)
            m_ts = []
            for ci in range(2):
                mbf = sm.tile([64, 512], bf16, tag=f"m{ci}bf")
                nc.vector.tensor_tensor(
                    mbf[:].rearrange("p (u j) -> p u j", j=16),
                    l4v[:, :, ci].unsqueeze(1).broadcast_to((64, 32, 16)),
                    umask.unsqueeze(2).broadcast_to((64, 32, 16)),
                    OP.mult)
                m_ts.append(mbf)

            s1view = s1bf[:]

            # ---- X build: per half, 2 accumulated matmuls
            xa = []
            for h in range(2):
                psX = psA.tile([128, 512], f32, tag=f"y{h}")
                nc.tensor.matmul(psX[:],
                                 s1view[32 * h:32 * h + 32],
                                 m_ts[0][32 * h:32 * h + 32, :],
                                 start=True, stop=False)
                nc.tensor.matmul(psX[:],
                                 s2bf[32 * h:32 * h + 32, :],
                                 m_ts[1][32 * h:32 * h + 32, :],
                                 start=False, stop=True)
                xt = xp.tile([128, 512], bf16, tag=f"x{h}")
                if h == 0:
                    nc.scalar.activation(xt[:], psX[:], AF.Copy)
                else:
                    nc.vector.tensor_copy(xt[:], psX[:])
                xa.append(xt)

            # ---- layers
            zb = [None, None]
            for l in range(N_LAYERS):
                for h in range(2):
                    yA = psA.tile([128, 512], f32, tag=f"y{h}")
                    nc.tensor.matmul(yA[:], W(2 * l), xa[h][:],
                                     start=True, stop=True)
                    yc = cv.tile([128, 512], bf16, tag=f"yc{h}")
                    nc.scalar.activation(yc[:], yA[:], AF.Copy)
                    xB = cv.tile([128, 512], bf16, tag=f"xb{h}")
                    nc.vector.transpose(xB[:], yc[:])
                    zB = psB.tile([128, 512], f32, tag=f"z{h}")
                    nc.tensor.matmul(zB[:], W(2 * l + 1), xB[:],
                                     start=True, stop=True)
                    if l < N_LAYERS - 1:
                        zc = cv.tile([128, 512], bf16, tag=f"zc{h}")
                        nc.scalar.activation(zc[:], zB[:], AF.Copy)
                        xt = xp.tile([128, 512], bf16, tag=f"x{h}")
                        nc.vector.transpose(xt[:], zc[:])
                        xa[h] = xt
                    else:
                        zb[h] = zB
                # one dummy MM per layer keeps the PE clock un-throttled
                # while ACT/DVE run the convert+transpose chain
                nc.tensor.matmul(wscr[:], W(0), wt[:, 0:512],
                                 start=True, stop=True,
                                 skip_group_check=True)

            # ---- measurement (layout B)
            outv = out_d.rearrange("(g t) q -> g t q", t=2)
            for h in range(2):
                sq = cv.tile([128, 512], bf16, tag=f"yc{h}")
                nc.scalar.square(sq[:], zb[h][:])
                o1 = psS.tile([32, 512], f32, tag=f"vp{h}")
                nc.tensor.matmul(o1[:], w1_t, sq[:], start=True, stop=True)
                o1c = cv.tile([32, 512], bf16, tag=f"xb{h}")
                nc.scalar.activation(o1c[:], o1[:], AF.Copy)
                o1t = cv.tile([32, 512], bf16, tag=f"zc{h}")
                nc.vector.transpose(o1t[:], o1c[:])
                o2 = psS.tile([8, 512], f32, tag="vT")
                nc.tensor.matmul(o2[:], w2_t, o1t[:], start=True, stop=True)
                res = sm.tile([8, 512], f32, tag=f"res{h}")
                nc.vector.tensor_copy(res[:], o2[:])
                # gather to out[b, q]; b = 32h + 2*bhi + b0
                resv = res[:].rearrange("p (u c) -> p u c", c=32)
                for b0 in range(2):
                    rows = outv[16 * h:16 * h + 16, b0]
                    eng = nc.sync if b0 == 0 else nc.scalar
                    # q5..q9 from row 0 (ones), cols b0*16+1..6
                    eng.dma_start(
                        rows[:, 5:10].unsqueeze(0),
                        resv[0:1, :, 16 * b0 + 1:16 * b0 + 6])
                    # q0..q4 from rows 1..5, col b0*16
                    eng.dma_start(
                        rows[:, 0:5].rearrange("u q -> q u"),
                        resv[1:6, :, 16 * b0])

    nc.finalize()
    return nc


def _get_module():
    if "nc" not in _BUILD_CACHE:
        _BUILD_CACHE["nc"] = _build_module()
    return _BUILD_CACHE["nc"]


# ---------------------------------------------------------------- entrypoint
def kernel(inputs, theta):
    inputs = np.asarray(inputs, dtype=np.float32)
    theta = np.asarray(theta, dtype=np.float32)
    assert inputs.shape == (B_TOTAL, N_QUBITS)

    from concourse.bass_utils import run_bass_kernel_spmd

    nc = _get_module()
    wstack = _host_weights(theta)
    in_maps = []
    for c in range(N_CORES):
        shard = np.ascontiguousarray(inputs[B_CORE * c:B_CORE * (c + 1)])
        in_maps.append({"xin": shard, "wstack": wstack})
    res = run_bass_kernel_spmd(nc, in_maps, core_ids=list(range(N_CORES)))
    out = np.concatenate([r["out"] for r in res.results], axis=0)
    return out.astype(np.float32)


# revision 41
# speedup vs baseline: 1.1472x; 1.1472x over previous
"""Trainium2 Bass kernel v2 for nn_EnhancedQuantumLayer (10-qubit, 4-layer
variational circuit, batch 512, Z-expectations output).

Scheme (data parallel over 8 cores, 64 samples/core):
  - Realified complex: re/im is a partition bit c. Layout A partitions
    p = (c, q5, q0..q4), free f = (b4..b1, b0, q6..q9) per half (b5).
    A 32x32 DVE block transpose flips to layout B p = (c, q5, b0, q6..q9),
    f = (b4..b1, q0..q4). One bf16 matmul per (layer, side, half).
  - Feature map: v(t) per (sample, qubit) is an exact 94-term Fourier
    series in t = tanh(x) (odd multiples of 0.25). Built on-device via
    partition_broadcast + per-partition tensor_scalar + range reduce +
    one ACT Sin + one PE contraction; un-transposed by 10 tiny PE
    transposes.
  - Initial state X = (h6 outer l4) built by 2 accumulated matmuls per
    half with the per-sample h6 amplitudes as stationary (k = samples).
  - Measurement: squares + two sign-weight matmuls, all in layout B.

Host precompute is theta-only: 8 realified 128x128 stationaries (bf16).
"""

import math

import ml_dtypes
import numpy as np

N_QUBITS = 10
N_LAYERS = 4
FREQS = (1.0, 2.0, 4.0, 8.0, 16.0)
PI = float(np.pi)
B_TOTAL = 512
B_CORE = 64
N_CORES = 8
WARMUP_MM = 5

CZCNOT = np.array([[1, 0, 0, 0],
                   [0, 1, 0, 0],
                   [0, 0, 0, -1],
                   [0, 0, 1, 0]], dtype=np.complex128)


# ---------------------------------------------------------------- host math
def _rz(phi):
    return np.array([[np.exp(-0.5j * phi), 0], [0, np.exp(0.5j * phi)]])


def _rx(th):
    c, s = np.cos(th / 2), np.sin(th / 2)
    return np.array([[c, -1j * s], [-1j * s, c]])


def _ry(th):
    c, s = np.cos(th / 2), np.sin(th / 2)
    return np.array([[c, -s], [s, c]])


def _kron_list(ms):
    out = ms[0]
    for m in ms[1:]:
        out = np.kron(out, m)
    return out


def _embed_2q(space_qubits, qa, qb, M4):
    n = len(space_qubits)
    dim = 2 ** n
    pa, pb = space_qubits.index(qa), space_qubits.index(qb)
    out = np.zeros((dim, dim), dtype=np.complex128)
    for idx in range(dim):
        bits = [(idx >> (n - 1 - i)) & 1 for i in range(n)]
        col4 = 2 * bits[pa] + bits[pb]
        for row4 in range(4):
            val = M4[row4, col4]
            if val != 0:
                nb = bits.copy()
                nb[pa], nb[pb] = row4 >> 1, row4 & 1
                ridx = sum(bit << (n - 1 - i) for i, bit in enumerate(nb))
                out[ridx, idx] += val
    return out


def _realify(M):
    return np.block([[M.real, -M.imag], [M.imag, M.real]])


def _embed_OB(M_L):
    """layout-B partition op on (q5, b0, q6..q9): identity on b0."""
    M = M_L.reshape(2, 16, 2, 16)
    O = np.zeros((2, 2, 16, 2, 2, 16), np.complex128)
    for b0 in range(2):
        O[:, b0, :, :, b0, :] = M
    return O.reshape(64, 64)


def _host_weights(theta):
    """wstack [128, 8, 128] bf16: per layer [lhsT_A, lhsT_L] with
    lhsT = realify(op).T, partition-major for one contiguous DMA."""
    ang = np.tanh(theta.astype(np.float64)) * PI
    A_space = [5, 0, 1, 2, 3, 4]
    L_space = [5, 6, 7, 8, 9]
    mats = []
    for l in range(N_LAYERS):
        U = []
        for q in range(10):
            a0, a1, a2 = ang[l, q]
            U.append(_rx(a0 * 0.5) @ _rz(a2) @ _ry(a1) @ _rz(a0))
        UA = _kron_list([U[q] for q in A_space])
        E_even_A = (_embed_2q(A_space, 0, 1, CZCNOT)
                    @ _embed_2q(A_space, 2, 3, CZCNOT)
                    @ _embed_2q(A_space, 4, 5, CZCNOT))
        E_odd_A = (_embed_2q(A_space, 1, 2, CZCNOT)
                   @ _embed_2q(A_space, 3, 4, CZCNOT))
        M_A = E_odd_A @ E_even_A @ UA
        UL = _kron_list([np.eye(2)] + [U[q] for q in [6, 7, 8, 9]])
        E_even_L = (_embed_2q(L_space, 6, 7, CZCNOT)
                    @ _embed_2q(L_space, 8, 9, CZCNOT))
        E_odd_L = (_embed_2q(L_space, 5, 6, CZCNOT)
                   @ _embed_2q(L_space, 7, 8, CZCNOT))
        M_L = E_odd_L @ E_even_L @ UL
        mats.append(_realify(M_A).T)
        mats.append(_realify(_embed_OB(M_L)).T)
    stk = np.stack(mats)  # [8, 128, 128]
    return np.ascontiguousarray(
        stk.transpose(1, 0, 2).astype(ml_dtypes.bfloat16))


# ------------------------------------------------------- fourier basis (v)
def _v_of_t(t):
    t = np.atleast_1d(np.asarray(t, np.float64))
    v = np.zeros((t.size, 2), np.complex128)
    v[:, 0] = 1.0
    for f in FREQS:
        phi = f * t
        v = v * np.stack([np.exp(-0.5j * phi), np.exp(0.5j * phi)], -1)
        th = 0.25 * f * t
        c, s = np.cos(th), np.sin(th)
        v = np.stack([c * v[:, 0] - 1j * s * v[:, 1],
                      -1j * s * v[:, 0] + c * v[:, 1]], -1)
    return v


def _fourier_C():
    """C [94, 4]: rows 0-46 sin(0.25 m t), rows 47-93 cos, m = 1,3..93;
    comps (ar, ai, br, bi)."""
    N = 1024
    ts = np.arange(N) * (8 * np.pi / N)
    vv = _v_of_t(ts)
    comps = np.stack([vv[:, 0].real, vv[:, 0].imag,
                      vv[:, 1].real, vv[:, 1].imag], -1)
    F = np.fft.rfft(comps, axis=0)
    msk = np.arange(1, 94, 2)
    a_cos = 2.0 * F[msk].real / N
    b_sin = -2.0 * F[msk].imag / N
    return msk, np.concatenate([b_sin, a_cos], 0).astype(np.float64)


# ------------------------------------------------------------- bass builder
_BUILD_CACHE = {}


def _measurement_weights():
    # W1 [128, 32]: col = b0p*16 + o; o: 0 = one, 1..5 = s5..s9
    W1 = np.zeros((128, 32), np.float32)
    for p in range(128):
        q5 = (p >> 5) & 1
        b0 = (p >> 4) & 1
        j4 = p & 15
        s = [1 - 2 * q5] + [1 - 2 * ((j4 >> (3 - k)) & 1) for k in range(4)]
        W1[p, b0 * 16 + 0] = 1.0
        for k in range(5):
            W1[p, b0 * 16 + 1 + k] = s[k]
    # W2 [32, 8]: m 0 = ones, 1..5 = sg_q0..sg_q4 over j5' = (q0..q4)
    W2 = np.zeros((32, 8), np.float32)
    for j in range(32):
        W2[j, 0] = 1.0
        for q in range(5):
            W2[j, 1 + q] = 1 - 2 * ((j >> (4 - q)) & 1)
    return W1, W2


def _build_module():
    import concourse.bass as bass
    import concourse.mybir as mybir
    from concourse import bacc
    from concourse.tile import TileContext

    f32 = mybir.dt.float32
    f32r = mybir.dt.float32r
    bf16 = mybir.dt.bfloat16
    AF = mybir.ActivationFunctionType
    OP = mybir.AluOpType

    nc = bacc.Bacc("TRN2", target_bir_lowering=False, debug=False)

    xin = nc.dram_tensor("xin", [B_CORE, 10], f32, kind="ExternalInput").ap()
    wstack = nc.dram_tensor("wstack", [128, 8, 128], bf16,
                            kind="ExternalInput").ap()
    out_d = nc.dram_tensor("out", [B_CORE, 10], f32, kind="ExternalOutput").ap()

    # ---- inline constants
    msk, C94 = _fourier_C()
    cpf = np.zeros((128, 16), np.float32)  # f32 consts: C, I4, signs
    cpf[0:94, 2:6] = C94
    cpf[0:4, 6:10] = np.eye(4)
    # scol [64, 2] signs; umask [64, 32]
    cpf[0:64, 10] = -1.0
    cpf[0:64, 11] = 1.0
    # cpwb [64, 256]: omega-broadcast and bias-broadcast stationaries for
    # the diagonal-spread args matmuls (all rows identical)
    om = np.zeros(128, np.float32)
    om[0:47] = 0.25 * msk
    om[47:94] = 0.25 * msk
    bi = np.zeros(128, np.float32)
    bi[47:94] = 0.5 * PI
    cpwb = np.zeros((64, 256), np.float32)
    cpwb[:, 0:128] = om
    cpwb[:, 128:256] = bi
    # dmask [64, 640]: delta(b, b') replicated over q (bias moving operand)
    dmask = np.zeros((64, 640), np.float32)
    for b in range(64):
        dmask[b, b::64] = 1.0
    # ci64: identity [64, 64] (diagonal spread mask)
    ci64 = np.eye(64, dtype=np.float32)
    cpm = np.zeros((64, 32), np.float32)
    for b in range(64):
        cpm[b, b % 32] = 1.0
    W1, W2 = _measurement_weights()
    cph = np.zeros((128, 32 + 8), np.float32)
    cph[:, 0:32] = W1
    cph[0:32, 32:40] = W2
    cpf_c = nc.inline_tensor(cpf, name="cpf").ap()
    cpwb_c = nc.inline_tensor(cpwb, name="cpwb").ap()
    dmask_c = nc.inline_tensor(dmask, name="dmask").ap()
    ci64_c = nc.inline_tensor(ci64, name="ci64").ap()
    cpm_c = nc.inline_tensor(cpm, name="cpm").ap()
    cph_c = nc.inline_tensor(cph.astype(ml_dtypes.bfloat16), name="cph").ap()

    MAGIC = 1.5 * 2 ** 23
    TWO_PI = 2.0 * PI
    PCLAMP = PI * (1.0 - 1e-6)

    with TileContext(nc) as tc:
        with (
            tc.tile_pool(name="wpool", bufs=1) as wpool,
            tc.tile_pool(name="sm", bufs=2) as sm,
            tc.tile_pool(name="dbl", bufs=2) as db,
            tc.tile_pool(name="xp", bufs=4) as xp,
            tc.tile_pool(name="cv", bufs=4) as cv,
            tc.tile_pool(name="psA", bufs=1, space="PSUM") as psA,
            tc.tile_pool(name="psB", bufs=1, space="PSUM") as psB,
            tc.tile_pool(name="psS", bufs=1, space="PSUM") as psS,
            tc.tile_pool(name="psW", bufs=1, space="PSUM") as psW,
        ):
            # ---- DMAs: consts for the first ACT op first, then inputs
            ctf = wpool.tile([128, 16], f32, tag="cpf")
            nc.sync.dma_start(ctf[:], cpf_c)
            sx = sm.tile([64, 10], f32, tag="sx")
            nc.sync.dma_start(sx[:], xin)
            ci64t = wpool.tile([64, 64], f32, tag="ci64")
            nc.sync.dma_start(ci64t[:], ci64_c)
            ctwb = wpool.tile([64, 256], f32, tag="cpwb")
            nc.sync.dma_start(ctwb[:], cpwb_c)
            dmt = wpool.tile([64, 640], f32, tag="dmask")
            nc.sync.dma_start(dmt[:], dmask_c)

            wt = wpool.tile([128, 8 * 128], bf16, tag="w")
            nc.scalar.dma_start(
                wt[:].rearrange("p (m c) -> p m c", c=128), wstack)
            ctm = wpool.tile([64, 32], f32, tag="cpm")
            nc.scalar.dma_start(ctm[:], cpm_c)
            cth = wpool.tile([128, 40], bf16, tag="cph")
            nc.scalar.dma_start(cth[:], cph_c)

            cmat = ctf[:, 2:6]
            ident4 = ctf[0:4, 6:10]
            scol = ctf[0:64, 10:12]
            umask = ctm[:]
            w1_t = cth[:, 0:32]
            w2_t = cth[0:32, 32:40]

            def W(m):
                return wt[:, 128 * m:128 * m + 128]

            # dummy Silu first: steers the act-table pass to load the one
            # table (silu_and_others) that serves Tanh+Sin+Copy+Square too
            sdum = sm.tile([1, 1], f32, tag="sdum")
            nc.scalar.activation(sdum[:], ctf[0:1, 0:1], AF.Silu)

            # ---- feature map: tanh on [64, 10]; diagonal-spread moving;
            # args = (omega-bcast)^T spread + (bias-bcast)^T dmask via PE
            tx = sm.tile([64, 10], f32, tag="tx")
            nc.scalar.activation(tx[:], sx[:], AF.Tanh)
            spr = sm.tile([64, 640], f32, tag="spr")
            nc.vector.tensor_tensor(
                spr[:].bitcast(f32r).rearrange("p (q b) -> p q b", b=64),
                tx[:].unsqueeze(2).broadcast_to((64, 10, 64)),
                ci64t[:].unsqueeze(1).broadcast_to((64, 10, 64)),
                OP.mult)
            argp = [None, None]
            for i in range(2):
                sl = slice(320 * i, 320 * (i + 1))
                ap_ = psA.tile([128, 320], f32, tag=f"y{i}")
                nc.tensor.matmul(ap_[:], ctwb[:, 0:128].bitcast(f32r),
                                 spr[:, sl].bitcast(f32r),
                                 start=True, stop=False)
                nc.tensor.matmul(ap_[:], ctwb[:, 128:256].bitcast(f32r),
                                 dmt[:, sl].bitcast(f32r),
                                 start=False, stop=True)
                argp[i] = ap_

            trig = sm.tile([128, 640], f32, tag="trig")
            for i in range(2):
                sl = slice(320 * i, 320 * (i + 1))
                kk = sm.tile([128, 320], f32, tag=f"kk{i}")
                nc.vector.tensor_scalar(kk[:], argp[i][:], 1.0 / TWO_PI,
                                        MAGIC, OP.mult, OP.add)
                nc.vector.tensor_scalar(kk[:], kk[:], MAGIC, None,
                                        OP.subtract)
                ar = sm.tile([128, 320], f32, tag=f"ar{i}")
                nc.vector.scalar_tensor_tensor(ar[:], kk[:], -TWO_PI,
                                               argp[i][:], OP.mult, OP.add)
                nc.vector.tensor_scalar(ar[:], ar[:], PCLAMP, -PCLAMP,
                                        OP.min, OP.max)
                nc.scalar.activation(trig[:, sl].bitcast(f32r), ar[:],
                                     AF.Sin)

            # ---- v = C^T trig : 2 matmuls then 10 tiny PE transposes
            vp0 = psS.tile([4, 320], f32, tag="vp0")
            vp1 = psS.tile([4, 320], f32, tag="vp1")
            vps = [vp0, vp1]
            for i in range(2):
                nc.tensor.matmul(vps[i][:], cmat.bitcast(f32r),
                                 trig[:, 320 * i:320 * (i + 1)].bitcast(f32r),
                                 start=True, stop=True)
            vsb = sm.tile([4, 640], f32, tag="vsb")
            for i in range(2):
                nc.vector.tensor_copy(
                    vsb[:, 320 * i:320 * (i + 1)].bitcast(f32r),
                    vps[i][:])
            vT = psS.tile([64, 40], f32, tag="vT")
            for q in range(10):
                nc.tensor.transpose(
                    vT[:, 4 * q:4 * q + 4].bitcast(f32r),
                    vsb[0:4, 64 * q:64 * q + 64].bitcast(f32r),
                    ident4.bitcast(f32r))
            v_cur = sm.tile([64, 40], f32, tag="vcur")
            nc.vector.tensor_copy(v_cur[:], vT[:])

            # ---- PE warm-keeper: WAW-chained f32 matmuls gated on v_cur so
            # the scheduler cannot hoist them; they fill the PE-idle gap
            # while the DVE doubling chain runs, keeping the HAM clock hot
            # into the layer loop.
            wscr = psW.tile([40, 512], f32, tag="warm")
            for i in range(WARMUP_MM):
                nc.tensor.matmul(wscr[:], v_cur[:], trig[0:64, 0:512],
                                 start=True, stop=True,
                                 skip_group_check=True)

            # ---- doubling: h6 = v5 x v0..v4 (j6 idx), l4 = v6 x..x v9
            vv = v_cur[:].rearrange("p (q a c) -> p q a c", a=2, c=2)
            vimS = sm.tile([64, 40], f32, tag="vimS")
            vimS_v = vimS[:].rearrange("p (q a c) -> p q a c", a=2, c=2)
            nc.vector.tensor_tensor(
                vimS_v,
                vv[:, :, :, 1:2].broadcast_to((64, 10, 2, 2)),
                scol.unsqueeze(1).unsqueeze(1)
                    .broadcast_to((64, 10, 2, 2))
                    .rearrange("p q a c -> p q a c"),
                OP.mult)

            def vre(q):
                return vv[:, q, :, 0]  # [64, 2]

            def vim_s(q):
                return vimS_v[:, q]  # [64, 2, 2]

            def kstep(eng, Xt, m, Yre, YimS, n, tag, dtype=f32, pool=db):
                """out[p, (j, a, c)] = complex (X kron Y); X [64, m*2]."""
                Xv = Xt[:].rearrange("p (m c) -> p m c", c=2)
                t1 = pool.tile([64, m * n * 2], f32, tag=tag + "1")
                t2 = pool.tile([64, m * n * 2], f32, tag=tag + "2")
                out = pool.tile([64, m * n * 2], dtype, tag=tag)
                t1v = t1[:].rearrange("p (m n c) -> p m n c", m=m, c=2)
                t2v = t2[:].rearrange("p (m n c) -> p m n c", m=m, c=2)
                Xb = Xv.unsqueeze(2).broadcast_to((64, m, n, 2))
                Xsw = (Xv[:, :, ::-1].unsqueeze(2)
                       .broadcast_to((64, m, n, 2)))
                Yreb = (Yre.unsqueeze(1).unsqueeze(3)
                        .broadcast_to((64, m, n, 2)))
                YimSb = YimS.unsqueeze(1).broadcast_to((64, m, n, 2))
                eng.tensor_tensor(t1v, Xb, Yreb, OP.mult)
                eng.tensor_tensor(t2v, Xsw, YimSb, OP.mult)
                eng.tensor_tensor(out[:], t1[:], t2[:], OP.add)
                return out

            def mk_imS(eng, Yt, n, tag, pool=db):
                o = pool.tile([64, n * 2], f32, tag=tag)
                ov = o[:].rearrange("p (n c) -> p n c", c=2)
                Yv = Yt[:].rearrange("p (n c) -> p n c", c=2)
                eng.tensor_tensor(
                    ov, Yv[:, :, 1:2].broadcast_to((64, n, 2)),
                    scol.unsqueeze(1).broadcast_to((64, n, 2)),
                    OP.mult)
                return o

            V = nc.vector
            # all chains on DVE (GPSIMD semaphore wake-up is ~5us; keep it
            # entirely off the critical path)
            v5t = sm.tile([64, 4], f32, tag="v5t")
            nc.vector.tensor_copy(
                v5t[:].rearrange("p (a c) -> p a c", c=2), vv[:, 5])
            a1 = kstep(V, v5t, 2, vre(0), vim_s(0), 2, "a1")
            a2 = kstep(V, a1, 4, vre(1), vim_s(1), 2, "a2")
            v6t = sm.tile([64, 4], f32, tag="v6t")
            nc.vector.tensor_copy(
                v6t[:].rearrange("p (a c) -> p a c", c=2), vv[:, 6])
            c1 = kstep(V, v6t, 2, vre(7), vim_s(7), 2, "c1")
            v2t = sm.tile([64, 4], f32, tag="v2t")
            nc.vector.tensor_copy(
                v2t[:].rearrange("p (a c) -> p a c", c=2), vv[:, 2])
            b1 = kstep(V, v2t, 2, vre(3), vim_s(3), 2, "b1")
            b2 = kstep(V, b1, 4, vre(4), vim_s(4), 2, "b2")
            b2S = mk_imS(V, b2, 8, "b2S")
            v8t = sm.tile([64, 4], f32, tag="v8t")
            nc.vector.tensor_copy(
                v8t[:].rearrange("p (a c) -> p a c", c=2), vv[:, 8])
            c2 = kstep(V, v8t, 2, vre(9), vim_s(9), 2, "c2")
            c2S = mk_imS(V, c2, 4, "c2S")
            # h6 = a2 x b2 (DVE) written c-major bf16 (stationary layout),
            # l4 = c1 x c2 (GPS)
            b2re = b2[:].rearrange("p (n c) -> p n c", c=2)[:, :, 0]
            b2Sv = b2S[:].rearrange("p (n c) -> p n c", c=2)
            h6t1 = db.tile([64, 128], f32, tag="h61")
            h6t2 = db.tile([64, 128], f32, tag="h62")
            t1v = h6t1[:].rearrange("p (m n c) -> p m n c", m=8, c=2)
            t2v = h6t2[:].rearrange("p (m n c) -> p m n c", m=8, c=2)
            a2v = a2[:].rearrange("p (m c) -> p m c", c=2)
            nc.vector.tensor_tensor(
                t1v, a2v.unsqueeze(2).broadcast_to((64, 8, 8, 2)),
                b2re.unsqueeze(1).unsqueeze(3).broadcast_to((64, 8, 8, 2)),
                OP.mult)
            nc.vector.tensor_tensor(
                t2v, a2v[:, :, ::-1].unsqueeze(2).broadcast_to((64, 8, 8, 2)),
                b2Sv.unsqueeze(1).broadcast_to((64, 8, 8, 2)),
                OP.mult)
            s1bf = sm.tile([64, 128], bf16, tag="s1bf")
            s1w = s1bf[:].rearrange("p (c m n) -> p m n c", c=2, m=8)
            nc.vector.tensor_tensor(
                s1w, t1v, t2v, OP.add)
            c2re = c2[:].rearrange("p (n c) -> p n c", c=2)[:, :, 0]
            l4 = kstep(V, c1, 4, c2re,
                       c2S[:].rearrange("p (n c) -> p n c", c=2), 4, "l4")

            # ---- S2 from S1 (c-major): S2[c] = sign(c) * S1[1-c]
            s1cm = s1bf[:].rearrange("p (c j) -> p c j", c=2)
            s2bf = sm.tile([64, 128], bf16, tag="s2bf")
            nc.vector.tensor_tensor(
                s2bf[:].rearrange("p (c j) -> p c j", c=2),
                s1cm[:, ::-1, :],
                scol.unsqueeze(2).broadcast_to((64, 2, 64)),
                OP.mult)

            l4v = l4[:].rearrange("p (j c) -> p j c", c=2)
            m_ts = []
            for ci in range(2):
                mbf = sm.tile([64, 512], bf16, tag=f"m{ci}bf")
                nc.vector.tensor_tensor(
                    mbf[:].rearrange("p (u j) -> p u j", j=16),
                    l4v[:, :, ci].unsqueeze(1).broadcast_to((64, 32, 16)),
                    umask.unsqueeze(2).broadcast_to((64, 32, 16)),
                    OP.mult)
                m_ts.append(mbf)

            s1view = s1bf[:]

            # ---- X build: per half, 2 accumulated matmuls
            xa = []
            for h in range(2):
                psX = psA.tile([128, 512], f32, tag=f"y{h}")
                nc.tensor.matmul(psX[:],
                                 s1view[32 * h:32 * h + 32],
                                 m_ts[0][32 * h:32 * h + 32, :],
                                 start=True, stop=False)
                nc.tensor.matmul(psX[:],
                                 s2bf[32 * h:32 * h + 32, :],
                                 m_ts[1][32 * h:32 * h + 32, :],
                                 start=False, stop=True)
                xt = xp.tile([128, 512], bf16, tag=f"x{h}")
                if h == 0:
                    nc.scalar.activation(xt[:], psX[:], AF.Copy)
                else:
                    nc.vector.tensor_copy(xt[:], psX[:])
                xa.append(xt)

            # ---- layers
            zb = [None, None]
            for l in range(N_LAYERS):
                for h in range(2):
                    yA = psA.tile([128, 512], f32, tag=f"y{h}")
                    nc.tensor.matmul(yA[:], W(2 * l), xa[h][:],
                                     start=True, stop=True)
                    yc = cv.tile([128, 512], bf16, tag=f"yc{h}")
                    nc.scalar.activation(yc[:], yA[:], AF.Copy)
                    xB = cv.tile([128, 512], bf16, tag=f"xb{h}")
                    nc.vector.transpose(xB[:], yc[:])
                    zB = psB.tile([128, 512], f32, tag=f"z{h}")
                    nc.tensor.matmul(zB[:], W(2 * l + 1), xB[:],
                                     start=True, stop=True)
                    if l < N_LAYERS - 1:
                        zc = cv.tile([128, 512], bf16, tag=f"zc{h}")
                        nc.scalar.activation(zc[:], zB[:], AF.Copy)
                        xt = xp.tile([128, 512], bf16, tag=f"x{h}")
                        nc.vector.transpose(xt[:], zc[:])
                        xa[h] = xt
                    else:
                        zb[h] = zB


            # ---- measurement (layout B)
            outv = out_d.rearrange("(g t) q -> g t q", t=2)
            for h in range(2):
                sq = cv.tile([128, 512], bf16, tag=f"yc{h}")
                nc.scalar.square(sq[:], zb[h][:])
                o1 = psS.tile([32, 512], f32, tag=f"vp{h}")
                nc.tensor.matmul(o1[:], w1_t, sq[:], start=True, stop=True)
                o1c = cv.tile([32, 512], bf16, tag=f"xb{h}")
                nc.scalar.activation(o1c[:], o1[:], AF.Copy)
                o1t = cv.tile([32, 512], bf16, tag=f"zc{h}")
                nc.vector.transpose(o1t[:], o1c[:])
                o2 = psS.tile([8, 512], f32, tag="vT")
                nc.tensor.matmul(o2[:], w2_t, o1t[:], start=True, stop=True)
                res = sm.tile([8, 512], f32, tag=f"res{h}")
                nc.vector.tensor_copy(res[:], o2[:])
                # gather to out[b, q]; b = 32h + 2*bhi + b0
                resv = res[:].rearrange("p (u c) -> p u c", c=32)
                for b0 in range(2):
                    rows = outv[16 * h:16 * h + 16, b0]
                    eng = nc.sync if b0 == 0 else nc.scalar
                    # q5..q9 from row 0 (ones), cols b0*16+1..6
                    eng.dma_start(
                        rows[:, 5:10].unsqueeze(0),
                        resv[0:1, :, 16 * b0 + 1:16 * b0 + 6])
                    # q0..q4 from rows 1..5, col b0*16
                    eng.dma_start(
                        rows[:, 0:5].rearrange("u q -> q u"),
                        resv[1:6, :, 16 * b0])

    nc.finalize()
    return nc


def _get_module():
    if "nc" not in _BUILD_CACHE:
        _BUILD_CACHE["nc"] = _build_module()
    return _BUILD_CACHE["nc"]


# ---------------------------------------------------------------- entrypoint
def kernel(inputs, theta):
    inputs = np.asarray(inputs, dtype=np.float32)
    theta = np.asarray(theta, dtype=np.float32)
    assert inputs.shape == (B_TOTAL, N_QUBITS)

    from concourse.bass_utils import run_bass_kernel_spmd

    nc = _get_module()
    wstack = _host_weights(theta)
    in_maps = []
    for c in range(N_CORES):
        shard = np.ascontiguousarray(inputs[B_CORE * c:B_CORE * (c + 1)])
        in_maps.append({"xin": shard, "wstack": wstack})
    res = run_bass_kernel_spmd(nc, in_maps, core_ids=list(range(N_CORES)))
    out = np.concatenate([r["out"] for r in res.results], axis=0)
    return out.astype(np.float32)
